# revision 11
# baseline (speedup 1.0000x reference)
"""Direct 3x3 valid conv (Winograd-equivalent output) on 8 TRN2 cores.

Problem: x [8, 64, 128, 128] f32, filt [64, 64, 3, 3] f32
         -> y [8, 64, 126, 126] f32  (valid correlation, stride 1)

Sharding: data-parallel, one sample per core.

Compute: row-pair-packed implicit GEMM. SBUF partitions hold (row-parity
tau, channel c): partitions 0-63 = even input rows, 64-127 = odd input
rows, of the same sample. PSUM partitions hold (out-row-parity q, k).
For an output row pair (2j, 2j+1), contributions come from input row
pairs j+u, u in {0,1}; tap dy = 2u + tau - q. Six weight matrices
W[u,dx][(tau,c),(q,k)] = filt[k,c,2u+tau-q,dx] (75% dense) replace the
9 block-diagonal (50% dense) per-tap matrices: 6 matmuls per 2 output
rows instead of 9 per 1 -> 1.5x fewer PE cycles at the same free size.

Everything device-side is bf16 (inputs cast on host, output upcast on
host); PSUM accumulates f32. Per chunk of 4 row pairs (free 4*126 =
504 <= 512 PSUM f32 bank), 6 matmuls accumulate one bank; chunk pairs
interleave taps across two banks so the PE never idles at group
boundaries; evict PSUM -> bf16 SBUF (vector) -> DMA out per 8 pairs.
"""

import numpy as np
import ml_dtypes

import concourse.bass as bass
import concourse.mybir as mybir
import concourse.tile as tile
from concourse import bacc
from concourse.bass_utils import run_bass_kernel_spmd

N, C, H, W = 8, 64, 128, 128
K = 64
OH = OW = H - 2            # 126
P = H // 2                 # 64 input row pairs
JP = OH // 2               # 63 output row pairs
N_CORES = 8
J_CHUNK = 4                # row pairs per PSUM chunk: 4*126 = 504 <= 512
N_CHUNKS = (JP + J_CHUNK - 1) // J_CHUNK   # 16
MATS = [(u, dx) for u in range(2) for dx in range(3)]
N_WARM = 6

BF16 = mybir.dt.bfloat16
NP_BF16 = ml_dtypes.bfloat16

_cache = {}


class _LightTileContext(tile.TileContext):
    """TileContext with a minimal end-of-kernel epilogue.

    The stock epilogue (sync.drain + all-engine barrier + gpsimd dma_reset/
    sem_clear + barrier) costs ~2.5us of sequencer semaphore chatter after
    the last useful instruction. The NEFF loader re-initializes semaphore
    state per execution, so for a single-shot kernel the terminal-value
    drain wait alone is sufficient: SP halts only after every producer has
    reached its final semaphore value (all DMAs landed, all engines done).
    """

    def _drain_and_barrier(self, tick_clock, wait_clock):
        nc = self.nc
        popped = nc._tile_sem_poison_stack.pop()
        assert popped is self._sem_poison
        d = nc.sync.drain()
        wait_clock.add_sem_waits(
            d.ins, tile.ScopedClock({None: tick_clock.global_clock})
        )


def _build_nc():
    nc = bacc.Bacc(None)
    xs = nc.dram_tensor("xs", [128, P, W], BF16, kind="ExternalInput")
    wt = nc.dram_tensor("wt", [128, 6, 128], BF16, kind="ExternalInput")
    out = nc.dram_tensor("out", [128, JP, OW], BF16, kind="ExternalOutput")

    with _LightTileContext(nc) as tc:
        with (
            tc.tile_pool(name="xpool", bufs=1) as xpool,
            tc.tile_pool(name="opool", bufs=3) as opool,
            tc.tile_pool(name="psum", bufs=6, space="PSUM") as psum,
        ):
            xs_sb = xpool.tile([128, P, W], BF16)
            wt_sb = xpool.tile([128, 6, 128], BF16, tag="wt_sb")
            warm_sb = xpool.tile([128, 512], BF16, tag="warm_sb")

            # Weights first (tiny, gates the first matmul), then banded input,
            # all on sync in FIFO order so band 0 lands right after wt.
            # Byte-range-precise deps let chunk c start once its bands landed.
            nc.sync.dma_start(wt_sb[:], wt[:])
            bands = [(0, 6)] + [(b, min(b + 8, P)) for b in range(6, P, 8)]
            for b0, b1 in bands:
                nc.sync.dma_start(xs_sb[:, b0:b1, :], xs[:, b0:b1, :])

            # PE warmup: dummy matmuls keep the PE busy through the HAM
            # activity window (clock ramp) while the input loads.
            nc.vector.memset(warm_sb[:], 0.0)
            warm_ps = psum.tile([128, 504], mybir.dt.float32, tag="warm_ps", bufs=1)
            for _ in range(N_WARM):
                nc.tensor.matmul(
                    warm_ps[:], warm_sb[:, 0:128], warm_sb[:, :504],
                    start=True, stop=True,
                )

            for cp in range(0, N_CHUNKS, 2):
                chunks = []
                pss = []
                for ci in (cp, cp + 1):
                    j0 = ci * J_CHUNK
                    jn = min(J_CHUNK, JP - j0)
                    chunks.append((j0, jn))
                    pss.append(
                        psum.tile(
                            [128, J_CHUNK, OW], mybir.dt.float32,
                            tag="ps", name=f"ps_{ci}",
                        )
                    )
                for mi, (u, dx) in enumerate(MATS):
                    for (j0, jn), ps in zip(chunks, pss):
                        nc.tensor.matmul(
                            ps[:, :jn, :],
                            wt_sb[:, mi, :],
                            xs_sb[:, j0 + u : j0 + u + jn, dx : dx + OW],
                            start=(mi == 0),
                            stop=(mi == len(MATS) - 1),
                        )
                # Evict each chunk on its own engine (vector / scalar) and
                # issue the out-DMA from the same queue: no cross-engine sem
                # between cast and store, and the two casts run in parallel.
                for hi, ((j0, jn), ps) in enumerate(zip(chunks, pss)):
                    ob = opool.tile([128, J_CHUNK, OW], BF16, tag=f"ob{hi}")
                    if hi == 0:
                        nc.vector.tensor_copy(ob[:, :jn, :], ps[:, :jn, :])
                        nc.sync.dma_start(out[:, j0 : j0 + jn, :], ob[:, :jn, :])
                    else:
                        nc.scalar.copy(ob[:, :jn, :], ps[:, :jn, :])
                        nc.scalar.dma_start(out[:, j0 : j0 + jn, :], ob[:, :jn, :])

    nc.finalize()
    return nc


def _shard_inputs(x, filt):
    # wt[tau*64+c, u*3+dx, q*64+k] = filt[k, c, 2u+tau-q, dx] (0 if dy invalid)
    filt = np.asarray(filt, dtype=np.float32)
    wt = np.zeros((128, 6, 128), dtype=np.float32)
    for u in range(2):
        for dx in range(3):
            m = u * 3 + dx
            for tau in range(2):
                for q in range(2):
                    dy = 2 * u + tau - q
                    if 0 <= dy <= 2:
                        wt[tau * 64:(tau + 1) * 64, m, q * 64:(q + 1) * 64] = (
                            filt[:, :, dy, dx].T
                        )
    wt = wt.astype(NP_BF16)

    # xb[s, tau*64+c, j, w] = x[s, c, 2j+tau, w]
    xb = np.asarray(x, dtype=np.float32).astype(NP_BF16)
    xb = np.ascontiguousarray(
        xb.reshape(N, C, P, 2, W).transpose(0, 3, 1, 2, 4)
    ).reshape(N, 128, P, W)

    return [{"xs": xb[s], "wt": wt} for s in range(N_CORES)]


def _gather(results):
    y = np.empty((N, K, OH, OW), dtype=np.float32)
    for s in range(N_CORES):
        o = np.asarray(results[s]["out"]).astype(np.float32)  # [(q,k), j, w]
        y[s] = o.reshape(2, K, JP, OW).transpose(1, 2, 0, 3).reshape(K, OH, OW)
    return y


def kernel(x, filt, **run_kwargs):
    if "nc" not in _cache:
        _cache["nc"] = _build_nc()
    in_maps = _shard_inputs(x, filt)
    res = run_bass_kernel_spmd(_cache["nc"], in_maps, list(range(N_CORES)), **run_kwargs)
    _cache["last_results"] = res
    return _gather(res.results)
